# revision 19
# baseline (speedup 1.0000x reference)
"""Malvar demosaic on 8 trn2 NeuronCores.

Input CFA [16,1,1024,1024] f32 + four 5x5 kernels -> output [16,3,1024,1024].

Strategy (pure data parallel, 2 images per core):
  - Each image is processed in 9 horizontal bands of 124 output rows.
  - The input band (incl. the +-2 row halo) is loaded parity-split in two
    row-strided DMAs: even rows r0-2..r0+124 at partitions [0:64], odd
    rows r0-1..r0+125 at [64:128].  Out-of-image halo rows are zero-filled
    from a tiny zeros input.
  - The entire Bayer demosaic (four 5x5 convs + per-pixel selection incl.
    the CFA pass-through) is folded into banded 128x126 float32r matmuls:
    for each (output channel, column parity) pair a composite lhsT applies
    the right conv's vertical taps (or the identity) per output row
    parity; the horizontal taps ride on stride-2 rhs slices of X and the
    +-1/+-2 horizontal tap sums S1/S2 (two vector-engine shifted adds).
    3 matmuls per (channel, col-parity) accumulate in one PSUM bank.
  - Each PSUM plane is evicted with a single dense copy into a packed
    [128, 3*1024] output tile (channels side by side), then two
    channel-merged row-parity-strided DMAs store each band.

Every compute-op partition slice starts at 0 or 64 (hard TRN2 engine-AP
constraint) and one lhsT matrix set works for every band (image-edge
zero-padding comes from the zero-filled halo partitions).
"""

import numpy as np

import concourse.bass as bass
import concourse.mybir as mybir
import concourse.tile as tile
from concourse.bass_utils import run_bass_kernel_spmd

B, H, W = 16, 1024, 1024
N_CORES = 8
IMGS_PER_CORE = B // N_CORES
BAND = 124              # output rows per band
NBANDS = (H + BAND - 1) // BAND   # 9
M = 126                 # matmul output partitions (evens [0:62], odds [64:126])
MM_DT = mybir.dt.float32r

# source per (channel, row-parity, col-parity): conv index 0..3 or "X"
_SEL = {
    (0, 0, 0): "X", (0, 0, 1): 1, (0, 1, 0): 2, (0, 1, 1): 3,   # R
    (1, 0, 0): 0, (1, 0, 1): "X", (1, 1, 0): "X", (1, 1, 1): 0,  # G
    (2, 0, 0): 3, (2, 0, 1): 2, (2, 1, 0): 1, (2, 1, 1): "X",    # B
}


def _build_matrices(k5s):
    """Packed lhsT [128, 18*M]: for each (channel, col-parity) a composite
    (X, S1, S2)-plane triple that applies the selected conv's vertical taps
    (or identity) per output-row parity."""
    packed = np.zeros((128, 18 * M), dtype=np.float32)

    def p_of(r):  # partition of absolute row r within the band tile of r0
        return None

    idx = 0
    for ch in range(3):
        for cp in range(2):
            Ms = [np.zeros((128, M), dtype=np.float32) for _ in range(3)]
            for d in range(BAND):                    # output row r0+d
                m = d // 2 if d % 2 == 0 else 64 + (d - 1) // 2
                src = _SEL[(ch, d % 2, cp)]
                if src == "X":
                    p = (d + 2) // 2 if d % 2 == 0 else 64 + (d + 1) // 2
                    Ms[0][p, m] += 1.0
                    continue
                k5 = k5s[src]
                assert np.allclose(k5[:, 1], k5[:, 3])
                assert np.allclose(k5[:, 0], k5[:, 4])
                for dy in range(-2, 3):
                    r = d + dy
                    p = (r + 2) // 2 if r % 2 == 0 else 64 + (r + 1) // 2
                    Ms[0][p, m] += k5[2 + dy, 2]
                    Ms[1][p, m] += k5[2 + dy, 1]
                    Ms[2][p, m] += k5[2 + dy, 0]
            for pl in range(3):
                packed[:, idx * M:(idx + 1) * M] = Ms[pl]
                idx += 1
    return packed


_CACHE = {}


def _split_waits(nc, max_waits=1):
    """The walrus in this container rejects instructions carrying more than
    one sem wait.  Hoist extra waits onto same-engine NoOps inserted right
    before the offending instruction (sequencer waits are executed in
    program order, so this is semantics-preserving)."""
    total = 0
    for bb in nc.main_func.blocks:
        insts = bb.bb.instructions if hasattr(bb, "bb") else bb.instructions
        i = 0
        while i < len(insts):
            ins = insts[i]
            si = ins.sync_info
            if si is not None and si.on_wait and len(si.on_wait) > max_waits:
                waits = list(si.on_wait)
                keep, hoist = waits[-max_waits:], waits[:-max_waits]
                nops = []
                for w in hoist:
                    nop = mybir.InstNoOp(
                        name=nc.get_next_instruction_name(),
                        engine=ins.engine, ins=[], outs=[],
                        sync_info=mybir.SyncInfo(on_wait=[w], on_update=[]))
                    nc.register_instruction(nop)
                    nops.append(nop)
                ins.sync_info = mybir.SyncInfo(
                    on_wait=keep, on_update=list(si.on_update or []))
                insts[i:i] = nops
                i += len(nops)
                total += len(nops)
            i += 1
    return total


def _build_nc():
    nc = bass.Bass(target_bir_lowering=False, trn_type="TRN2")
    x = nc.dram_tensor("x", [IMGS_PER_CORE, 1, H, W], MM_DT,
                       kind="ExternalInput")
    wts = nc.dram_tensor("wm", [128, 18 * M], MM_DT, kind="ExternalInput")
    zpad = nc.dram_tensor("zpad", [1, W], MM_DT, kind="ExternalInput")
    out = nc.dram_tensor("out", [IMGS_PER_CORE, 3, H, W], mybir.dt.float32,
                         kind="ExternalOutput")

    with tile.TileContext(nc) as tc:
        with (
            tc.tile_pool(name="wpool", bufs=1) as wpool,
            tc.tile_pool(name="xpool", bufs=4) as xpool,
            tc.tile_pool(name="spool", bufs=4) as spool,
            tc.tile_pool(name="opool", bufs=4) as opool,
            tc.tile_pool(name="psum", bufs=1, space="PSUM") as pspool,
        ):
            wt = wpool.tile([128, 18 * M], MM_DT)
            nc.scalar.dma_start(wt[:], wts[:])

            for b in range(IMGS_PER_CORE):
                for t in range(NBANDS):
                    r0 = t * BAND
                    n_rows = min(BAND, H - r0)       # stored rows this band
                    n_me = (n_rows + 1) // 2
                    n_mo = n_rows // 2
                    # alternate HWDGE rings per band: loads on one, stores on
                    # the other, swapping each band to balance both rings
                    ld_eng = nc.scalar if (t + b) % 2 == 0 else nc.sync
                    st_eng = nc.sync if (t + b) % 2 == 0 else nc.scalar

                    xt = xpool.tile([128, W + 4], MM_DT, tag="x")
                    # evens r0-2..r0+124 -> [0:64], odds r0-1..r0+125 -> [64:128]
                    for par in range(2):
                        lo, hi = r0 - 2 + par, r0 + BAND + par + 1
                        vlo = lo if lo >= 0 else lo + 2   # keep row parity
                        vhi = min(hi, H)
                        p0 = par * 64 + (vlo - lo) // 2
                        cnt = (vhi - vlo + 1) // 2
                        ld_eng.dma_start(
                            xt[p0:p0 + cnt, 2:W + 2],
                            x[b, 0, vlo:vhi:2, :])
                        if lo < 0:      # first band: halo rows above image
                            ld_eng.dma_start(
                                xt[par * 64:par * 64 + 1, 2:W + 2], zpad[:, :])
                        if hi > H:      # last band: first row past the image
                            ld_eng.dma_start(
                                xt[p0 + cnt:p0 + cnt + 1, 2:W + 2], zpad[:, :])
                    nc.gpsimd.memset(xt[:, 0:2].bitcast(mybir.dt.float32), 0.0)
                    nc.gpsimd.memset(
                        xt[:, W + 2:W + 4].bitcast(mybir.dt.float32), 0.0)

                    s1 = spool.tile([128, W], MM_DT, tag="s1")
                    s2 = spool.tile([128, W], MM_DT, tag="s2")
                    nc.vector.tensor_tensor(
                        s1[:], xt[:, 1:W + 1], xt[:, 3:W + 3], mybir.AluOpType.add)
                    nc.vector.tensor_tensor(
                        s2[:], xt[:, 0:W], xt[:, 4:W + 4], mybir.AluOpType.add)

                    plane = opool.tile([128, 3 * W], mybir.dt.float32, tag="pl")

                    for ci, (ch, cp) in enumerate(
                            (c, p) for c in range(3) for p in range(2)):
                        ps = pspool.tile([M, 512], mybir.dt.float32,
                                         tag=f"ps{ci}", name=f"ps{ci}")
                        rhss = (xt[:, 2 + cp:2 + cp + W:2],
                                s1[:, cp:W:2], s2[:, cp:W:2])
                        for pl in range(3):
                            nc.tensor.matmul(
                                ps[:],
                                wt[:, (ci * 3 + pl) * M:(ci * 3 + pl + 1) * M],
                                rhss[pl],
                                start=(pl == 0), stop=(pl == 2))
                        dst = plane[0:M, ch * W + cp:ch * W + W:2]
                        if ci % 2 == 0:
                            nc.vector.tensor_copy(dst, ps[:])
                        else:
                            nc.scalar.copy(dst, ps[:])

                    # two channel-merged row-parity stores
                    for par, p0, cnt in ((0, 0, n_me), (1, 64, n_mo)):
                        st_eng.dma_start(
                            out[b, :, r0 + par:r0 + n_rows:2, :].rearrange(
                                "c h w -> h c w"),
                            plane[p0:p0 + cnt, :].rearrange(
                                "p (c w) -> p c w", c=3))

    _split_waits(nc)
    nc.finalize()
    return nc


def _get_nc():
    if "nc" not in _CACHE:
        _CACHE["nc"] = _build_nc()
    return _CACHE["nc"]


def kernel(CFA_inputs, GR_GB, Rg_RB_Bg_BR, Rg_BR_Bg_RB, Rb_BB_Br_RR, _trace=False):
    cfa = np.ascontiguousarray(np.asarray(CFA_inputs, dtype=np.float32))
    k5s = [np.asarray(k, dtype=np.float32)
           for k in (GR_GB, Rg_RB_Bg_BR, Rg_BR_Bg_RB, Rb_BB_Br_RR)]
    nc = _get_nc()

    wm = _build_matrices(k5s)
    zpad = np.zeros((1, W), dtype=np.float32)
    in_maps = [{"x": cfa[c * IMGS_PER_CORE:(c + 1) * IMGS_PER_CORE],
                "wm": wm, "zpad": zpad} for c in range(N_CORES)]

    res = run_bass_kernel_spmd(nc, in_maps, core_ids=list(range(N_CORES)),
                               trace=_trace)
    outs = np.concatenate([res.results[c]["out"] for c in range(N_CORES)], axis=0)
    if _trace:
        kernel._last = res
    return outs
